# revision 101
# baseline (speedup 1.0000x reference)
"""AERGCN (2-layer R-GCN + bilinear attention pool) on 8 TRN2 NeuronCores.

Sharding: pair-hybrid. Cores are paired (2p, 2p+1); pair p owns batches
A=2p, B=2p+1. Within a pair the 41 relations split 20/20 (even core: rels
0-19, odd: 20-39) and relation 40 is computed by BOTH cores at half weight
(the 0.5 is folded into the relation-softmax exp bias as +ln 0.5), keeping
the graph fully SPMD-symmetric. Each layer runs one stream of 42 (rel,
batch) combos strictly alternating A/B (A leads by 2) so each weight tile
is DMA'd once and consumed by both batches back-to-back, keeping weight
demand at a steady 1 tile / 2 combos. Per layer, ONE pairwise AllReduce
([2,S,769] bf16: full-A | full-B payloads) reconstitutes the relation
softmax for both batches in a single collective (fixed CC cost ~11us paid
once per layer, not twice). 1/denom is precomputed on host, so combos
issue no GpSimd work and CC triggers can't head-of-line-block the pipe.
After layer 2 each core runs the attention pool for its own batch
(selected from the AllReduce output by a data-driven mask to stay SPMD).

Matmuls in bf16 (f32 PSUM). Per-combo pipeline:
  hidden = h @ [W_r | W_r @ score_w]          (12 accumulating matmuls)
  logun = adj @ u                             (1 matmul, N=1, lhsT=adjT)
  e = exp(logun*rec + bias); scr = e*rec      (rec = 1/denom from host)
  payload += scr * (adj @ hidden)             (2 matmuls N=384; DVE)
"""

import os
import sys

# The Bass NEFF executes through the axon PJRT backend; if the caller pinned
# jax to cpu before we ever import jax, lift the pin so axon devices resolve.
if "jax" not in sys.modules and os.environ.get("JAX_PLATFORMS") == "cpu":
    os.environ["JAX_PLATFORMS"] = ""

import numpy as np
import ml_dtypes

bf16 = ml_dtypes.bfloat16

B, S, F, R, NL = 8, 128, 768, 41, 2
NH, HD, EMB = 8, 96, 768
NCORES, IC = 8, 6
FE = F + 1      # 769: W with appended u column
RSLOT = 21      # 20 private relations + shared relation 40 (half weight)
LEAD = 2        # batch A runs this many relations ahead of batch B

_CACHE = {}


def _build_graph():
    if "nc" in _CACHE:
        return _CACHE["nc"]

    import concourse.mybir as mybir
    import concourse.tile as tile
    from concourse import bacc
    from concourse.masks import make_identity

    dt = mybir.dt
    AF = mybir.ActivationFunctionType
    OP = mybir.AluOpType

    nc = bacc.Bacc("TRN2", target_bir_lowering=False, debug=False,
                   num_devices=NCORES)

    # ---------------- DRAM I/O (per-core shapes) ----------------
    # all big tensors pre-permuted on host so every DMA is a straight
    # [partition, contiguous-bytes] copy (no strided descriptors).
    xt2 = nc.dram_tensor("xt2", [2, 128, IC * S], dt.bfloat16,
                         kind="ExternalInput")
    adjt_d = nc.dram_tensor("adjt", [RSLOT, 128, 2 * S], dt.bfloat16,
                            kind="ExternalInput")
    wa_d = nc.dram_tensor("wa", [NL, RSLOT, 128, IC * 512], dt.bfloat16,
                          kind="ExternalInput")
    wb_d = nc.dram_tensor("wb", [NL, RSLOT, 128, IC * 257], dt.bfloat16,
                          kind="ExternalInput")
    rec_d = nc.dram_tensor("rec", [S, 2 * RSLOT], dt.float32,
                           kind="ExternalInput")
    ebias_d = nc.dram_tensor("ebias", [S, 2 * NL], dt.float32,
                             kind="ExternalInput")
    mask_d = nc.dram_tensor("mask", [S, 2], dt.float32, kind="ExternalInput")
    wk_d = nc.dram_tensor("wk", [F, F], dt.bfloat16, kind="ExternalInput")
    wproj_d = nc.dram_tensor("wproj", [F, F], dt.bfloat16, kind="ExternalInput")
    bk_d = nc.dram_tensor("bk", [1, F], dt.bfloat16, kind="ExternalInput")
    bproj_d = nc.dram_tensor("bproj", [1, F], dt.bfloat16, kind="ExternalInput")
    # wkq = wk @ block_diag(qw per head)  [768, 8] and bkq = per-head bias
    # (host-precomputed: the whole attention q-side is input-only math, so
    # score[s,h] = hf @ wkq + bkq directly from the transposed features)
    wkq_d = nc.dram_tensor("wkq", [128, IC * NH], dt.bfloat16,
                           kind="ExternalInput")
    bkq_d = nc.dram_tensor("bkq", [1, NH], dt.bfloat16,
                           kind="ExternalInput")
    out_d = nc.dram_tensor("out", [1, F], dt.float32, kind="ExternalOutput")

    PAIRS = [[0, 1], [2, 3], [4, 5], [6, 7]]

    with tile.TileContext(nc) as tc:
        with (
            tc.tile_pool(name="const", bufs=1) as constp,
            tc.tile_pool(name="wpool", bufs=6) as wpool,
            tc.tile_pool(name="adjp", bufs=1) as adjp,
            tc.tile_pool(name="hidp", bufs=4) as hidp,
            tc.tile_pool(name="hT", bufs=1) as hTp,
            tc.tile_pool(name="payl", bufs=1) as paylp,
            tc.tile_pool(name="tail", bufs=4) as tailp,
            tc.tile_pool(name="misc", bufs=1) as miscp,
            tc.tile_pool(name="dram", bufs=1, space="DRAM") as dramp,
            tc.tile_pool(name="ps_hid", bufs=2, space="PSUM") as ps_hid,
            tc.tile_pool(name="ps_ld", bufs=2, space="PSUM") as ps_ld,
            tc.tile_pool(name="ps_intm", bufs=2, space="PSUM") as ps_intm,
        ):
            # layer-1 lhsT first in program order: the first combo needs it
            cur_hT = {}
            t = hTp.tile([128, IC * S], dt.bfloat16, name="hT0")
            nc.sync.dma_start(t[:], xt2[0])
            cur_hT[0] = t

            adj_tiles = {}
            w_cache = {}

            gate = {}

            def load_w(l, r):
                if (l, r) not in w_cache:
                    ta = wpool.tile([128, IC * 512], dt.bfloat16,
                                    name=f"wa{l}_{r}", tag="wta")
                    tb = wpool.tile([128, IC * 257], dt.bfloat16,
                                    name=f"wb{l}_{r}", tag="wtb")
                    if l == 1 and 1 <= r <= 6 and "t" in gate:
                        # hold these prefetches until the boundary collective
                        # lands so its DMA path is uncontended (WAW dep)
                        nc.vector.tensor_copy(ta[0:1, 0:1], gate["t"][0:1, 0:1])
                        nc.vector.tensor_copy(tb[0:1, 0:1], gate["t"][0:1, 0:1])
                    nc.sync.dma_start(ta[:], wa_d[l, r])
                    nc.sync.dma_start(tb[:], wb_d[l, r])
                    w_cache[(l, r)] = (ta, tb)
                return w_cache[(l, r)]

            def get_adjT(r, j):
                if r not in adj_tiles:
                    t = adjp.tile([S, 2 * S], dt.bfloat16, name=f"adjT{r}")
                    nc.sync.dma_start(t[:], adjt_d[r])
                    adj_tiles[r] = t
                return adj_tiles[r][:, j * S:(j + 1) * S]

            # first combos' data ahead of all constant/warmup traffic;
            # the very first weight tile is DMA'd per 512-col chunk so the
            # first matmul starts as soon as chunk 0 lands
            ta0 = wpool.tile([128, IC * 512], dt.bfloat16, name="wa0_0",
                             tag="wta")
            for ic in range(IC):
                nc.sync.dma_start(ta0[:, ic * 512:(ic + 1) * 512],
                                  wa_d[0, 0, :, ic * 512:(ic + 1) * 512])
            tb0 = wpool.tile([128, IC * 257], dt.bfloat16, name="wb0_0",
                             tag="wtb")
            nc.sync.dma_start(tb0[:], wb_d[0, 0])
            w_cache[(0, 0)] = (ta0, tb0)
            get_adjT(0, 0)
            t = hTp.tile([128, IC * S], dt.bfloat16, name="hT1")
            nc.sync.dma_start(t[:], xt2[1])
            cur_hT[1] = t
            load_w(0, 1)
            get_adjT(1, 0)

            ident_b = constp.tile([128, 128], dt.bfloat16, name="ident_b")
            make_identity(nc, ident_b)
            # dummy matmul burst while the first weight DMA streams: the PE
            # HAM clock-gate warms up (~3.4us of activity) so the first real
            # combos run at 2.4GHz instead of 1.2
            warm_ps = ps_hid.tile([S, 512], dt.float32, name="warm_ps",
                                  tag="hid")
            for _ in range(56):
                nc.tensor.matmul(warm_ps[:, 0:128], lhsT=ident_b[:],
                                 rhs=ident_b[:], start=True, stop=True)
            ones_row = constp.tile([1, 128], dt.bfloat16, name="ones_row")
            nc.vector.memset(ones_row, 1.0)
            one_sb = constp.tile([1, 1], dt.bfloat16, name="one_sb")
            nc.vector.memset(one_sb, 1.0)
            rec_sb = constp.tile([S, 2 * RSLOT], dt.float32, name="rec_sb")
            nc.sync.dma_start(rec_sb[:], rec_d[:])
            wkq_sb = constp.tile([128, IC * NH], dt.bfloat16, name="wkq_sb")
            nc.sync.dma_start(wkq_sb[:], wkq_d[:])
            bkq_sb = constp.tile([1, NH], dt.bfloat16, name="bkq_sb")
            nc.sync.dma_start(bkq_sb[:], bkq_d[:])
            ebias_sb = constp.tile([S, 2 * NL], dt.float32, name="ebias_sb")
            nc.sync.dma_start(ebias_sb[:], ebias_d[:])
            mask_sb = constp.tile([S, 2], dt.float32, name="mask_sb")
            nc.sync.dma_start(mask_sb[:], mask_d[:])

            # collective bounce buffers (DRAM pool so Tile tracks deps)
            warm_in = dramp.tile([8, 16], dt.bfloat16, name="warm_in")
            warm_out = dramp.tile([8, 16], dt.bfloat16, name="warm_out")
            arin = [dramp.tile([2, S, FE], dt.bfloat16, name=f"arin{l}")
                    for l in range(NL)]
            arout = [dramp.tile([2, S, FE], dt.bfloat16, name=f"arout{l}")
                     for l in range(NL)]

            # warm up the CC rings before the first real collective
            warm_sb = constp.tile([8, 16], dt.bfloat16, name="warm_sb")
            nc.vector.memset(warm_sb, 1.0)
            nc.sync.dma_start(warm_in[:], warm_sb[:])
            nc.gpsimd.collective_compute(
                "AllReduce", OP.add, replica_groups=PAIRS,
                ins=[warm_in.opt()], outs=[warm_out.opt()])

            payload = {}
            denacc = {}
            pycs = {}
            pend = [None]

            def rest(l, r, j, hid, adjT):
                col = 2 * r + j
                # second half first: its last column is adj @ (h@u) = logun,
                # so the score chain starts while the first half streams
                intm2 = ps_intm.tile([S, 385], dt.float32,
                                     name=f"in{l}{r}{j}1", tag="intm")
                nc.tensor.matmul(intm2[:], lhsT=adjT, rhs=hid[:, 384:FE],
                                 start=True, stop=True)
                # e = exp(logun*rec + bias); scr = e*rec
                tmul = tailp.tile([S, 1], dt.float32, name=f"tm{l}{r}{j}",
                                  tag="tm")
                nc.vector.tensor_mul(tmul[:], intm2[:, 384:385],
                                     rec_sb[:, col:col + 1])
                bcol = 2 * l + (1 if r == RSLOT - 1 else 0)
                ee = tailp.tile([S, 1], dt.float32, name=f"ee{l}{r}{j}",
                                tag="ee")
                nc.scalar.activation(ee[:], tmul[:], AF.Exp,
                                     bias=ebias_sb[:, bcol:bcol + 1])
                scr = tailp.tile([S, 1], dt.float32, name=f"sc{l}{r}{j}",
                                 tag="sc")
                nc.vector.tensor_mul(scr[:], ee[:], rec_sb[:, col:col + 1])
                first = (l, j) not in payload
                last = r == RSLOT - 1
                if first:
                    payload[(l, j)] = paylp.tile([S, FE], dt.float32,
                                                 name=f"pay{l}_{j}")
                    denacc[(l, j)] = tailp.tile([S, 1], dt.float32,
                                                name=f"den{l}{j}", bufs=1)
                pay = payload[(l, j)]
                if last:
                    # final accumulate writes the bf16 CC staging buffer
                    # directly, so shipping is just a DMA (no convert copy)
                    pyc = miscp.tile([S, FE], dt.bfloat16, name=f"pyc{l}{j}",
                                     tag="pyc", bufs=2)
                    pycs[(l, j)] = pyc
                    nc.vector.tensor_add(pyc[:, F:FE], denacc[(l, j)][:],
                                         ee[:])
                elif first:
                    nc.vector.tensor_copy(denacc[(l, j)][:], ee[:])
                else:
                    nc.vector.tensor_add(denacc[(l, j)][:], denacc[(l, j)][:],
                                         ee[:])
                intm1 = ps_intm.tile([S, 384], dt.float32,
                                     name=f"in{l}{r}{j}0", tag="intm")
                nc.tensor.matmul(intm1[:], lhsT=adjT, rhs=hid[:, 0:384],
                                 start=True, stop=True)
                for intm, c0, cw in ((intm2, 384, 384), (intm1, 0, 384)):
                    dst = pycs[(l, j)][:, c0:c0 + cw] if last \
                        else pay[:, c0:c0 + cw]
                    if first:
                        nc.vector.tensor_scalar(dst, intm[:, 0:cw], scr[:],
                                                None, OP.mult)
                    else:
                        nc.vector.scalar_tensor_tensor(dst, intm[:, 0:cw],
                                                       scr[:],
                                                       pay[:, c0:c0 + cw],
                                                       OP.mult, OP.add)
                return (l, r, j)

            def combo(l, r, j):
                """Emit transform of (l,r,j); flush the PREVIOUS combo's
                aggregation behind it (software pipeline)."""
                wta, wtb = load_w(l, r)
                adjT = get_adjT(r, j)
                hid_ps = ps_hid.tile([S, FE], dt.float32,
                                     name=f"hps{l}_{r}_{j}", tag="hid")
                for wt, cw, c0, c1 in ((wta, 512, 0, 512),
                                       (wtb, 257, 512, FE)):
                    for ic in range(IC):
                        nc.tensor.matmul(
                            hid_ps[:, c0:c1],
                            lhsT=cur_hT[j][:, ic * S:(ic + 1) * S],
                            rhs=wt[:, ic * cw:(ic + 1) * cw],
                            start=(ic == 0), stop=(ic == IC - 1))
                hid = hidp.tile([S, FE], dt.bfloat16,
                                name=f"hid{l}_{r}_{j}", tag="hid")
                # upper half (with u column) first: rest() consumes it first
                nc.scalar.copy(hid[:, 384:FE], hid_ps[:, 384:FE])
                nc.scalar.copy(hid[:, 0:384], hid_ps[:, 0:384])
                prev = pend[0]
                pend[0] = (l, r, j, hid, adjT)
                if prev is not None:
                    return rest(*prev)
                return None

            def flush():
                prev = pend[0]
                pend[0] = None
                if prev is not None:
                    return rest(*prev)
                return None

            def ship_half(l, j):
                """Ship the batch's payload (already staged in bf16)."""
                nc.sync.dma_start(arin[l][j][:], pycs[(l, j)][:])

            def ship(l):
                """One merged pairwise AllReduce: [full-A | full-B]."""
                ship_half(l, 1)
                nc.gpsimd.collective_compute(
                    "AllReduce", OP.add, replica_groups=PAIRS,
                    ins=[arin[l].opt()], outs=[arout[l].opt()])

            def h2_prep(l, j):
                # two half-DMAs land on separate queue lanes; the upper half
                # (with the denominator column) first so the reciprocal and
                # relu chain starts while the lower half streams
                raw = miscp.tile([S, FE], dt.bfloat16, name=f"raw{l}{j}",
                                 tag="raw", bufs=2)
                nc.sync.dma_start(raw[:, 384:FE], arout[l][j][:, 384:FE])
                nc.sync.dma_start(raw[:, 0:384], arout[l][j][:, 0:384])
                gate["t"] = raw
                rd = miscp.tile([S, 1], dt.float32, name=f"rd{l}{j}", tag="rd")
                nc.vector.reciprocal(rd[:], raw[:, F:FE])
                h2 = miscp.tile([S, F], dt.bfloat16, name=f"h2_{l}{j}",
                                tag="h2")
                t = hTp.tile([128, IC * S], dt.bfloat16, name=f"h2T{l}{j}")
                # chunked relu->transpose so the next layer's first matmul
                # can start as soon as chunk 0 lands
                for ic in range(IC):
                    sl = slice(ic * 128, (ic + 1) * 128)
                    nc.scalar.activation(h2[:, sl], raw[:, sl], AF.Relu,
                                         scale=rd[:])
                    tp = ps_ld.tile([128, 128], dt.bfloat16,
                                    name=f"tp{l}{j}_{ic}", tag="ld")
                    nc.tensor.transpose(tp[:], h2[:, sl], ident_b[:])
                    nc.scalar.copy(t[:, ic * S:(ic + 1) * S], tp[:])
                cur_hT[j] = t

            # attention weight tiles (loaded during layer 2)
            att = {}

            def _load_att_weights():
                bk_sb = constp.tile([1, F], dt.bfloat16, name="bk_sb")
                nc.sync.dma_start(bk_sb[:], bk_d[:])
                bp_sb = constp.tile([1, F], dt.bfloat16, name="bp_sb")
                nc.sync.dma_start(bp_sb[:], bproj_d[:])
                wkts, wpts = [], []
                for ic in range(IC):
                    wkt = wpool.tile([128, F], dt.bfloat16, name=f"wk{ic}",
                                     tag="wkt", bufs=IC)
                    nc.sync.dma_start(wkt[:], wk_d[ic * 128:(ic + 1) * 128, :])
                    wkts.append(wkt)
                    wpt = wpool.tile([128, F], dt.bfloat16, name=f"wp{ic}",
                                     tag="wpt", bufs=IC)
                    nc.sync.dma_start(wpt[:],
                                      wproj_d[ic * 128:(ic + 1) * 128, :])
                    wpts.append(wpt)
                att["bk"] = bk_sb
                att["bp"] = bp_sb
                att["wk"] = wkts
                att["wp"] = wpts

            def layer(l, hooks):
                seqA = [(r, 0) for r in range(RSLOT)]
                seqB = [(r, 1) for r in range(RSLOT)]
                seq = []
                ia = ib = 0
                while ia < len(seqA) or ib < len(seqB):
                    if ia < len(seqA) and (ia - ib < LEAD or ib >= len(seqB)):
                        seq.append(seqA[ia])
                        ia += 1
                    else:
                        seq.append(seqB[ib])
                        ib += 1
                last_a = (l, RSLOT - 1, 0)
                for k, (r, j) in enumerate(seq):
                    if k in hooks:
                        hooks[k]()
                    if combo(l, r, j) == last_a:
                        ship_half(l, 0)   # A done early: overlap its staging
                done = flush()
                if done == last_a:
                    ship_half(l, 0)
                ship(l)

            layer(0, {})
            h2_prep(0, 0)
            layer(1, {1: lambda: h2_prep(0, 1), 6: _load_att_weights})

            # =================== attention (own batch via mask) ==========
            r0 = miscp.tile([S, FE], dt.bfloat16, name="r0", tag="raw", bufs=2)
            nc.sync.dma_start(r0[:], arout[1][0][:])
            r1 = miscp.tile([S, FE], dt.bfloat16, name="r1", tag="raw", bufs=2)
            nc.sync.dma_start(r1[:], arout[1][1][:])
            # denominator first, then masked-combine + relu + transpose per
            # 128-col chunk so downstream matmuls start on chunk 0 early
            dcol = miscp.tile([S, 1], dt.float32, name="dcol", tag="rd2")
            nc.vector.tensor_scalar(dcol[:], r0[:, F:FE], mask_sb[:, 0:1],
                                    None, OP.mult)
            nc.vector.scalar_tensor_tensor(dcol[:], r1[:, F:FE],
                                           mask_sb[:, 1:2], dcol[:],
                                           OP.mult, OP.add)
            rd = miscp.tile([S, 1], dt.float32, name="rdf", tag="rd")
            nc.vector.reciprocal(rd[:], dcol[:])
            rawf = miscp.tile([S, F], dt.float32, name="rawf", tag="rawf")
            hf = miscp.tile([S, F], dt.bfloat16, name="hf", tag="h2")
            hfT = hTp.tile([128, IC * S], dt.bfloat16, name="hfT")
            for ic in range(IC):
                sl = slice(ic * 128, (ic + 1) * 128)
                nc.vector.tensor_scalar(rawf[:, sl], r0[:, sl],
                                        mask_sb[:, 0:1], None, OP.mult)
                nc.vector.scalar_tensor_tensor(rawf[:, sl], r1[:, sl],
                                               mask_sb[:, 1:2], rawf[:, sl],
                                               OP.mult, OP.add)
                nc.scalar.activation(hf[:, sl], rawf[:, sl], AF.Relu,
                                     scale=rd[:])
                tp = ps_ld.tile([128, 128], dt.bfloat16, name=f"ftp{ic}",
                                tag="ld")
                nc.tensor.transpose(tp[:], hf[:, sl], ident_b[:])
                nc.scalar.copy(hfT[:, ic * S:(ic + 1) * S], tp[:])

            bk_sb, bp_sb = att["bk"], att["bp"]
            wkts, wpts = att["wk"], att["wp"]
            # kx = hf @ wk + bk   [S, 768]  (bias folded via ones_row matmul)
            kx_ps = ps_hid.tile([S, F], dt.float32, name="kx_ps", tag="hid")
            for ic in range(IC):
                lhsT = hfT[:, ic * S:(ic + 1) * S]
                nc.tensor.matmul(kx_ps[:, 0:512], lhsT=lhsT,
                                 rhs=wkts[ic][:, 0:512],
                                 start=(ic == 0), stop=False)
                nc.tensor.matmul(kx_ps[:, 512:F], lhsT=lhsT,
                                 rhs=wkts[ic][:, 512:F],
                                 start=(ic == 0), stop=False)
            nc.tensor.matmul(kx_ps[:, 0:512], lhsT=ones_row[:],
                             rhs=bk_sb[:, 0:512], start=False, stop=True)
            nc.tensor.matmul(kx_ps[:, 512:F], lhsT=ones_row[:],
                             rhs=bk_sb[:, 512:F], start=False, stop=True)
            kx = miscp.tile([S, F], dt.bfloat16, name="kx", tag="kx")

            # scoreT[h,s] = (hf @ wkq + bkq).T computed directly in [8,S]
            # layout (lhsT = wkq chunks), so softmax needs no transpose
            scT_ps = ps_intm.tile([NH, S], dt.float32, name="scT_ps",
                                  tag="intm")
            for c in range(IC):
                nc.tensor.matmul(scT_ps[:],
                                 lhsT=wkq_sb[:, c * NH:(c + 1) * NH],
                                 rhs=hfT[:, c * S:(c + 1) * S],
                                 start=(c == 0), stop=False)
            nc.tensor.matmul(scT_ps[:], lhsT=bkq_sb[:], rhs=ones_row[:],
                             start=False, stop=True)
            # softmax without max-subtraction (scores are O(10) bilinear
            # forms, exp stays in f32 range)
            esc = miscp.tile([NH, S], dt.float32, name="esc", tag="esc", bufs=2)
            sumexp = miscp.tile([NH, 1], dt.float32, name="sumexp", tag="sm", bufs=3)
            nc.scalar.activation(esc[:], scT_ps[:], AF.Exp,
                                 accum_out=sumexp[:])
            # kx convert queued behind exp on the scalar engine: it overlaps
            # the DVE softmax steps instead of delaying the exp
            nc.scalar.copy(kx[:], kx_ps[:])
            rsm = miscp.tile([NH, 1], dt.float32, name="rsm", tag="sm", bufs=3)
            nc.vector.reciprocal(rsm[:], sumexp[:])
            attn = miscp.tile([NH, S], dt.bfloat16, name="attn", tag="esc", bufs=2)
            nc.vector.tensor_scalar_mul(attn[:], esc[:], rsm[:])
            # attnT [128, 8]
            at_ps = ps_ld.tile([S, NH], dt.bfloat16, name="at_ps", tag="ld")
            nc.tensor.transpose(at_ps[:], attn[:], ident_b[:NH, :NH])
            attnT = miscp.tile([S, NH], dt.bfloat16, name="attnT", tag="scb", bufs=3)
            nc.scalar.copy(attnT[:], at_ps[:])
            # oT[p, c] = o[128c+p] = attn_h . kx[:, 128c+p] computed directly
            # in transposed layout: 12 partition-sliced N=1 matmuls (heads
            # span 96-wide ranges, 32-aligned pieces across 128-chunks)
            oT_ps = ps_intm.tile([128, IC], dt.float32, name="oT_ps",
                                 tag="intm")
            for hh in range(NH):
                g0, g1 = hh * HD, (hh + 1) * HD
                while g0 < g1:
                    c, p0 = divmod(g0, 128)
                    p1 = min(g1 - c * 128, 128)
                    # legal (base, rows) for PE col-tiling: base 0 -> any,
                    # base 64 -> <=64, base 32/96 -> <=32
                    lim = 128 if p0 == 0 else (64 if p0 == 64 else 32)
                    p1 = min(p1, p0 + lim)
                    nc.tensor.matmul(oT_ps[p0:p1, c:c + 1],
                                     lhsT=kx[:, c * 128 + p0:c * 128 + p1],
                                     rhs=attnT[:, hh:hh + 1],
                                     start=True, stop=True,
                                     tile_position=(0, p0))
                    g0 = c * 128 + p1
            oT = miscp.tile([S, IC], dt.bfloat16, name="oT", tag="scb", bufs=3)
            nc.scalar.copy(oT[:], oT_ps[:])
            # res = o @ wproj + bproj
            res_ps = ps_hid.tile([1, 512], dt.float32, name="res_ps",
                                 tag="hid")
            res_ps2 = ps_hid.tile([1, 256], dt.float32, name="res_ps2",
                                  tag="hid")
            for ic in range(IC):
                nc.tensor.matmul(res_ps[:], lhsT=oT[:, ic:ic + 1],
                                 rhs=wpts[ic][:, 0:512],
                                 start=(ic == 0), stop=False)
                nc.tensor.matmul(res_ps2[:], lhsT=oT[:, ic:ic + 1],
                                 rhs=wpts[ic][:, 512:F],
                                 start=(ic == 0), stop=False)
            nc.tensor.matmul(res_ps[:], lhsT=one_sb[:], rhs=bp_sb[:, 0:512],
                             start=False, stop=True)
            nc.tensor.matmul(res_ps2[:], lhsT=one_sb[:], rhs=bp_sb[:, 512:F],
                             start=False, stop=True)
            res_sb = miscp.tile([1, F], dt.float32, name="res_sb", tag="res")
            nc.scalar.copy(res_sb[:, 0:512], res_ps[:])
            nc.scalar.copy(res_sb[:, 512:F], res_ps2[:])
            nc.sync.dma_start(out_d[:], res_sb[:])

    nc.compile()
    _CACHE["nc"] = nc
    return nc


def _prep_inputs(x, adj, q, w_rgcn, score_w, score_b, wk, bk, wq, bq, wbil,
                 wproj, bproj):
    f32 = np.float32
    x = np.asarray(x, f32)
    adj = np.asarray(adj, f32)
    q = np.asarray(q, f32)
    w_rgcn = np.asarray(w_rgcn, f32)
    score_w = np.asarray(score_w, f32)
    score_b = np.asarray(score_b, f32)

    u = np.einsum("lrio,lo->lri", w_rgcn, score_w).astype(f32)
    w_ext = np.concatenate([w_rgcn, u[..., None]], axis=-1)  # [2,41,768,769]
    # pre-permute: [l, r, i, f] -> [l, r, p, (c f)] with i = c*128 + p,
    # split into the two matmul column halves for finer DMA granularity
    w_pcf = w_ext.reshape(NL, R, IC, 128, FE).transpose(0, 1, 3, 2, 4)
    wa_perm = np.ascontiguousarray(
        w_pcf[..., 0:512].reshape(NL, R, 128, IC * 512)).astype(bf16)
    wb_perm = np.ascontiguousarray(
        w_pcf[..., 512:FE].reshape(NL, R, 128, IC * 257)).astype(bf16)

    # adj rowsum reciprocal (device never computes denominators)
    den = adj.sum(axis=3)                                    # [B, R, S]
    rec = (1.0 / np.where(den == 0.0, 1.0, den)).astype(f32)

    shared = {
        "wk": np.asarray(wk, f32).astype(bf16),
        "wproj": np.asarray(wproj, f32).astype(bf16),
        "bk": np.asarray(bk, f32).reshape(1, F).astype(bf16),
        "bproj": np.asarray(bproj, f32).reshape(1, F).astype(bf16),
    }

    # attention q-side is input-only math: qw = (q@wq+bq per head) @ wbil
    qx = q @ np.asarray(wq, f32) + np.asarray(bq, f32)       # [B, 768]
    qw = np.einsum("bhd,de->bhe", qx.reshape(B, NH, HD),
                   np.asarray(wbil, f32))                    # [B, NH, HD]
    # wkq[i, h] = sum_e wk[i, 96h+e] * qw[h, e]; bkq[h] likewise from bk
    wk_f = np.asarray(wk, f32).reshape(F, NH, HD)
    wkq = np.einsum("ihe,bhe->bih", wk_f, qw)                # [B, 768, 8]
    bkq = np.einsum("he,bhe->bh", np.asarray(bk, f32).reshape(NH, HD), qw)
    # device layout: wkq_sb[p, c*8+h] = wkq[c*128+p, h]
    wkq_dev = np.ascontiguousarray(
        wkq.reshape(B, IC, 128, NH).transpose(0, 2, 1, 3)
        .reshape(B, 128, IC * NH)).astype(bf16)
    bkq_dev = bkq.reshape(B, 1, NH).astype(bf16)

    in_maps = []
    for c in range(NCORES):
        p, h = c // 2, c % 2
        A, Bb = 2 * p, 2 * p + 1
        rsel = list(range(0, 20) if h == 0 else range(20, 40)) + [R - 1]
        adjt_c = np.empty((RSLOT, 128, 2 * S), f32)
        rec_c = np.empty((S, 2 * RSLOT), f32)
        for j, bb in enumerate((A, Bb)):
            adjt_c[:, :, j * S:(j + 1) * S] = adj[bb, rsel].transpose(0, 2, 1)
            rec_c[:, 2 * np.arange(RSLOT) + j] = rec[bb, rsel].T
        eb_c = np.empty((S, 2 * NL), f32)
        for l in range(NL):
            eb_c[:, 2 * l] = score_b[l]
            eb_c[:, 2 * l + 1] = score_b[l] + np.log(0.5)
        mask_c = np.zeros((S, 2), f32)
        mask_c[:, h] = 1.0
        xh = np.empty((2, 128, IC * S), f32)
        for j, bb in enumerate((A, Bb)):
            xh[j] = (x[bb].T.reshape(IC, 128, S).transpose(1, 0, 2)
                     .reshape(128, IC * S))
        m = dict(shared)
        m["wa"] = np.ascontiguousarray(wa_perm[:, rsel])
        m["wb"] = np.ascontiguousarray(wb_perm[:, rsel])
        m["adjt"] = np.ascontiguousarray(adjt_c).astype(bf16)
        m["rec"] = rec_c
        m["ebias"] = eb_c
        m["mask"] = mask_c
        m["wkq"] = wkq_dev[c]
        m["bkq"] = bkq_dev[c]
        m["xt2"] = xh.astype(bf16)
        in_maps.append(m)
    return in_maps


def kernel(**inputs) -> np.ndarray:
    from concourse.bass_utils import run_bass_kernel_spmd

    nc = _build_graph()
    in_maps = _prep_inputs(**inputs)
    res = run_bass_kernel_spmd(nc, in_maps, core_ids=list(range(NCORES)))
    outs = [np.asarray(res.results[c]["out"], np.float32) for c in range(NCORES)]
    return np.stack(outs)  # [8, 1, 768]
